# revision 36
# baseline (speedup 1.0000x reference)
"""GNN message-passing net on 8 Trainium2 cores — batch-pipelined v2.

Reference: x:[256,784,1] -> h1 = elu(spmm(x)@W1+b1) -> h2 = elu(spmm(h1)@W2+b2)
-> flat[B, N*C] -> relu(flat@Wf1+bf1) -> softmax(z@Wf2+bf2).

Strategy (all matmul operands bf16, fp32 PSUM accumulation):
  * Densify A (784x784, ~1% nz) on host; spmm becomes dense matmuls.
  * conv1 outer product: y = A @ X^T [784,256] replicated per core;
    h1_c = elu(W1[c]*y+b1[c]) for this core's 4 channels (elu via
    Exp/Relu ACT passes with fused scale/bias + 2 DVE ops).
  * conv2 channel-sharded: core k computes A @ h1_c for channels
    4k..4k+4, batch-half free dim.
  * AllToAll (per batch half) reshards channel->node; W2 mix as a
    kron(I4,W2) stationary matmul; +b2, elu.
  * FC1 K-sharded over nodes with h2 chunks as the STATIONARY operand:
    z_half[128b, 512h] accumulates 28 chunks per batch half.
  * One merged ReduceScatter over [256,512] (identity batch split:
    half 0 = cores 0-3's outputs): flat 32-row block k IS core k's
    output slice. Tail: PE-transpose z, +bf1 relu, FC2, softmax, once.
  * The two batch halves are software-pipelined: collectives (TOPSP/
    SDMA silicon) overlap compute; the conv side finishes well before
    the collective runtime's one-time init barrier (~40us, absorbed by
    a tiny AllGather alongside its ~12us first-op tax) lifts.
"""
import json

import numpy as np

import concourse.bass as bass
import concourse.mybir as mybir
import concourse.tile as tile
from concourse.bass_utils import run_bass_kernel_spmd

B, N, F, E = 256, 784, 1, 6272
C, H, N_OUT = 32, 512, 10
NCORE = 8
CPC = C // NCORE      # 4 channels per core in conv2
P = 112               # 784 = 7 * 112
KN = N // P           # 7 node chunks
NG = 4                # node groups packed into partitions for the mix
NS = P // NG          # 28 nodes per group per core
HJ = H // 128         # 4 h chunks
BH = B // 2           # batch half (pipeline stage)
BQ = 16               # output rows per core per half
NHALF = 2

f32 = mybir.dt.float32
bf16 = mybir.dt.bfloat16
AF = mybir.ActivationFunctionType
ALU = mybir.AluOpType
AX = mybir.AxisListType


# ---------------------------------------------------------------------------
# BIR post-pass: this walrus build rejects instructions with >1 sync-wait;
# split extras onto standalone EventSemaphore instructions (same engine,
# inserted just before, so the engine stream stalls identically).
def _split_waits(bir: dict, max_waits: int = 1) -> dict:
    n = [0]
    for fn in bir.get("functions", []):
        for blk in fn.get("blocks", []):
            out = []
            for ins in blk.get("instructions", []):
                si = ins.get("sync_info") or {}
                waits = si.get("on_wait") or []
                if len(waits) > max_waits:
                    for w in waits[max_waits:]:
                        n[0] += 1
                        out.append({
                            "name": f"I-waitsplit-{n[0]}",
                            "opcode": "EventSemaphore",
                            "engine": ins["engine"],
                            "ins": [], "outs": [],
                            **({"debug": ins["debug"]} if "debug" in ins else {}),
                            "sync_info": {"on_update": [], "on_wait": [w]},
                        })
                    si = dict(si)
                    si["on_wait"] = waits[:max_waits]
                    ins = dict(ins)
                    ins["sync_info"] = si
                out.append(ins)
            blk["instructions"] = out
    return bir


def _install_wait_splitter(nc):
    orig = nc.to_json_bytes
    nc.to_json_bytes = lambda: json.dumps(_split_waits(json.loads(orig()))).encode()


# ---------------------------------------------------------------------------
def _build_program():
    nc = bass.Bass(num_devices=NCORE)

    at_d = nc.dram_tensor("at", [P, KN * N], bf16, kind="ExternalInput")
    xt_d = nc.dram_tensor("xt", [P, KN * B], bf16, kind="ExternalInput")
    wf1_d = nc.dram_tensor("wf1", [NS * 128, H], bf16, kind="ExternalInput")
    wb_d = nc.dram_tensor("wb", [1, 2 * CPC], f32, kind="ExternalInput")
    w2k_d = nc.dram_tensor("w2k", [128, 128], bf16, kind="ExternalInput")
    b2k_d = nc.dram_tensor("b2k", [128, 1], f32, kind="ExternalInput")
    bf1t_d = nc.dram_tensor("bf1t", [128, HJ], f32, kind="ExternalInput")
    wf2_d = nc.dram_tensor("wf2", [128, HJ * N_OUT], bf16, kind="ExternalInput")
    bf2_d = nc.dram_tensor("bf2", [1, N_OUT], bf16, kind="ExternalInput")
    eye_d = nc.dram_tensor("eye16", [32, 32], bf16, kind="ExternalInput")
    out_d = nc.dram_tensor("out", [NHALF * BQ, N_OUT], f32, kind="ExternalOutput")

    with tile.TileContext(nc) as tc:
        with (
            tc.tile_pool(name="big", bufs=1) as big,
            tc.tile_pool(name="work", bufs=1) as work,
            tc.tile_pool(name="ps", bufs=1, space="PSUM") as ps,
            tc.tile_pool(name="dram", bufs=1, space="DRAM") as dram,
        ):
            # ---- resident tiles ------------------------------------------
            # at_sb is m-major: [p, mo, kc, mw] = A^T[kc*112+p, mo*112+mw]
            at_sb = big.tile([P, KN, KN, P], bf16)
            xt_sb = big.tile([P, KN, B], bf16)
            wf1_sb = big.tile([128, NS, H], bf16)
            y_sb = big.tile([P, KN, B], f32)
            wb_sb = work.tile([1, 2 * CPC], f32)
            w2k_sb = work.tile([128, 128], bf16)
            b2k_sb = work.tile([128, 1], f32)
            bf1t_sb = work.tile([128, HJ], f32)
            wf2_sb = work.tile([128, HJ, N_OUT], bf16)
            bf2_sb = work.tile([1, N_OUT], bf16)
            eye_sb = work.tile([32, 32], bf16)
            ones16 = work.tile([1, NHALF * BQ], bf16)
            ones_f = work.tile([1, 128], f32)
            warm_sb = work.tile([128, 512], bf16)
            zpad_sb = work.tile([P, CPC, BH], bf16)
            dummy_sb = work.tile([1, 16], f32)
            wband = work.tile([128, 2 * CPC], f32)

            # ---- t0: memsets + input loads (at/xt critical on sync; the
            # small weights go on gpsimd; the big wf1 is deferred to the
            # scalar queue later — it is only needed by FC1) ---------------
            nc.vector.memset(ones16[:], 1.0)
            nc.vector.memset(ones_f[:], 1.0)
            nc.vector.memset(warm_sb[:], 0.03)
            nc.vector.memset(zpad_sb[:], 0.0)
            nc.vector.memset(dummy_sb[:], 0.0)

            # tiny AllGather absorbs the one-time collective-runtime init
            # (~40us barrier + ~12us first-op tax) while compute proceeds
            dummy_in = dram.tile([1, 16], f32)
            dummy_out = dram.tile([NCORE, 16], f32, addr_space="Shared")
            nc.gpsimd.dma_start(dummy_in[:], dummy_sb[:])
            nc.gpsimd.collective_compute(
                "AllGather", ALU.bypass,
                replica_groups=[list(range(NCORE))],
                ins=[dummy_in.opt()], outs=[dummy_out.opt()],
            )

            nc.sync.dma_start(xt_sb[:],
                              xt_d[:].rearrange("p (k b) -> p k b", k=KN))
            at_ap = at_d[:].rearrange("p (m k w) -> p m k w", m=KN, k=KN)
            for mo in range(KN):
                nc.sync.dma_start(at_sb[:, mo, :, :], at_ap[:, mo, :, :])
            nc.gpsimd.dma_start(wb_sb[:], wb_d[:])
            nc.gpsimd.dma_start(w2k_sb[:], w2k_d[:])
            nc.gpsimd.dma_start(b2k_sb[:], b2k_d[:])
            nc.gpsimd.dma_start(bf1t_sb[:], bf1t_d[:])
            nc.gpsimd.dma_start(wf2_sb[:],
                                wf2_d[:].rearrange("p (j o) -> p j o", j=HJ))
            nc.gpsimd.dma_start(bf2_sb[:], bf2_d[:])
            nc.gpsimd.dma_start(eye_sb[:], eye_d[:])

            # ---- TE warmup (keeps HAM at 8/8 while inputs load) ----------
            for w in range(20):
                wp = ps.tile([128, 8, BH], f32, tag="pm", bufs=2,
                              name=f"warm{w}")
                nc.tensor.matmul(wp[:, 0:4, :], warm_sb[:, 0:128],
                                 warm_sb[:])

            # ---- broadcast W1/b1 channel scalars across partitions -------
            ps_bc = ps.tile([128, BQ], f32, tag="zt", bufs=2)
            nc.tensor.matmul(ps_bc[:, 0:2 * CPC], ones_f[0:1, 0:128], wb_sb[:])
            nc.vector.tensor_copy(wband[:], ps_bc[:, 0:2 * CPC])

            # ---- conv1 (both halves, free=256) ---------------------------
            for mo in range(KN):
                yp = ps.tile([P, B], f32, tag="mm1", bufs=2,
                             name=f"yp{mo}")
                for kc in range(KN):
                    nc.tensor.matmul(
                        yp[:], at_sb[:, mo, kc, :], xt_sb[:, kc, :],
                        start=(kc == 0), stop=(kc == KN - 1),
                    )
                nc.vector.tensor_copy(y_sb[:, mo, :], yp[:])

            # ---- per-half stage builders ---------------------------------
            h1_t = [None] * NHALF
            out2_t = [None] * NHALF
            a2ain_t = [None] * NHALF
            a2aout_t = [None] * NHALF
            r_t = [None] * NHALF
            h2_t = [None] * NHALF
            z_t = [None] * NHALF

            def elu1(h):
                h1 = big.tile([P, CPC, KN, BH], bf16, tag="h1", bufs=2,
                              name=f"h1_{h}")
                h1_t[h] = h1
                ysl = y_sb[:, :, h * BH:(h + 1) * BH]
                for c in range(CPC):
                    sc = wband[0:P, c:c + 1]
                    bi = wband[0:P, CPC + c:CPC + c + 1]
                    e = work.tile([P, KN, BH], bf16, tag="ew", bufs=3,
                                  name=f"e1_{h}{c}")
                    r = work.tile([P, KN, BH], bf16, tag="rw", bufs=3,
                                  name=f"r1_{h}{c}")
                    nc.scalar.activation(e[:], ysl, AF.Exp, bias=bi, scale=sc)
                    if c % 2:
                        # relu path on DVE to balance the engines
                        t = work.tile([P, KN, BH], f32, tag="tw", bufs=2,
                                      name=f"t1_{h}{c}")
                        nc.vector.tensor_scalar(t[:], ysl, sc, bi,
                                                ALU.mult, ALU.add)
                        nc.vector.tensor_scalar(r[:], t[:], 0.0, None,
                                                ALU.max)
                    else:
                        nc.scalar.activation(r[:], ysl, AF.Relu,
                                             bias=bi, scale=sc)
                    nc.vector.tensor_scalar(e[:], e[:], 1.0, -1.0,
                                            ALU.min, ALU.add)
                    nc.vector.tensor_tensor(h1[:, c, :, :], e[:], r[:],
                                            ALU.add)

            def conv2(h):
                h1 = h1_t[h]
                # [p, (mo, cl), b] so the a2a send merges (mo, cl) cleanly
                out2 = big.tile([P, KN * CPC, BH], bf16, tag="o2", bufs=2,
                                name=f"out2_{h}")
                out2_t[h] = out2
                for mo in range(KN):
                    o2 = ps.tile([P, CPC, BH], f32, tag="mm1", bufs=2,
                                 name=f"o2p_{h}{mo}")
                    for kc in range(KN):
                        nc.tensor.matmul(
                            o2[:], at_sb[:, mo, kc, :], h1[:, :, kc, :],
                            start=(kc == 0), stop=(kc == KN - 1),
                        )
                    dst = out2[:, mo * CPC:(mo + 1) * CPC, :]
                    if mo % 2:
                        nc.scalar.copy(dst, o2[:])
                    else:
                        nc.vector.tensor_copy(dst, o2[:])

            def a2a_send(h):
                a2ain = dram.tile([NCORE, CPC * P, BH], bf16, tag="a2ai",
                                  bufs=2, name=f"a2ain_{h}")
                a2aout = dram.tile([NCORE, CPC * P, BH], bf16, tag="a2ao",
                                   bufs=2, name=f"a2aout_{h}")
                a2ain_t[h], a2aout_t[h] = a2ain, a2aout
                dst = a2ain[0:KN].rearrange("j (cl p) b -> p (j cl) b",
                                            cl=CPC)
                nc.sync.dma_start(dst[:, 0:16, :], out2_t[h][:, 0:16, :])
                nc.sync.dma_start(dst[:, 16:28, :], out2_t[h][:, 16:28, :])
                nc.gpsimd.dma_start(
                    a2ain[KN].rearrange("(cl p) b -> p cl b", cl=CPC),
                    zpad_sb[:])
                nc.gpsimd.collective_compute(
                    "AllToAll", ALU.bypass,
                    replica_groups=[list(range(NCORE))],
                    ins=[a2ain.opt()], outs=[a2aout.opt()],
                )

            def a2a_recv(h):
                r_sb = big.tile([128, NS, BH], bf16, tag="r", bufs=2,
                                name=f"r_{h}")
                r_t[h] = r_sb
                sap = a2aout_t[h][:].rearrange(
                    "k (cl ng s) b -> ng (k cl) s b", cl=CPC, ng=NG)
                engs = [nc.gpsimd, nc.scalar, nc.sync, nc.gpsimd]
                for ng in range(NG):
                    engs[ng].dma_start(r_sb[ng * C:(ng + 1) * C, 0:8, :],
                                       sap[ng][:, 0:8, :])
                for ng in range(NG):
                    engs[ng].dma_start(r_sb[ng * C:(ng + 1) * C, 8:NS, :],
                                       sap[ng][:, 8:NS, :])

            def mix_fc1(h):
                r_sb = r_t[h]
                h2 = big.tile([128, NS, BH], bf16, tag="h2", bufs=2,
                              name=f"h2_{h}")
                h2_t[h] = h2
                zp = ps.tile([128, H], f32, tag="zt", bufs=2, name=f"zp_{h}")

                def mix_pair(g):
                    # two 4-slot mix blocks share one 2-bank psum tile so
                    # the elu ops run at FD=1024 (halves instr overhead)
                    s0 = 8 * g
                    sw = min(8, NS - s0)
                    pm = ps.tile([128, 8, BH], f32, tag="pm", bufs=2,
                                 name=f"pm_{h}{g}")
                    nc.tensor.matmul(pm[:, 0:4, :], w2k_sb[:],
                                     r_sb[:, s0:s0 + 4, :])
                    if sw == 8:
                        nc.tensor.matmul(pm[:, 4:8, :], w2k_sb[:],
                                         r_sb[:, s0 + 4:s0 + 8, :])
                    pv = pm[:, 0:sw, :]
                    e = work.tile([128, 8, BH], bf16, tag="e2", bufs=3,
                                  name=f"e2_{h}{g}")
                    r = work.tile([128, 8, BH], bf16, tag="r2", bufs=3,
                                  name=f"r2_{h}{g}")
                    nc.scalar.activation(e[:, 0:sw, :], pv, AF.Exp,
                                         bias=b2k_sb[:, 0:1])
                    if g % 2:
                        nc.vector.tensor_scalar(r[:, 0:sw, :], pv,
                                                b2k_sb[:, 0:1], 0.0,
                                                ALU.add, ALU.max)
                    else:
                        nc.scalar.activation(r[:, 0:sw, :], pv, AF.Relu,
                                             bias=b2k_sb[:, 0:1])
                    nc.vector.tensor_scalar(e[:, 0:sw, :], e[:, 0:sw, :],
                                            1.0, -1.0, ALU.min, ALU.add)
                    nc.vector.tensor_tensor(h2[:, s0:s0 + sw, :],
                                            e[:, 0:sw, :], r[:, 0:sw, :],
                                            ALU.add)

                def fc1_run(s0, s1):
                    for s in range(s0, s1):
                        nc.tensor.matmul(
                            zp[:], h2[:, s, :], wf1_sb[:, s, :],
                            start=(s == 0), stop=(s == NS - 1),
                        )

                mix_pair(0)          # s 0..8
                mix_pair(1)          # s 8..16
                fc1_run(0, 8)
                mix_pair(2)          # s 16..24
                fc1_run(8, 16)
                mix_pair(3)          # s 24..28
                fc1_run(16, 24)
                fc1_run(24, 28)

                z_sb = work.tile([128, H], bf16, tag="z", bufs=2,
                                 name=f"z_{h}")
                z_t[h] = z_sb
                nc.vector.tensor_copy(z_sb[:], zp[:])

            rs_buf = [None, None]

            def rs_push(h):
                # both halves land in one [256, H] buffer; one RS hands
                # core k its 32 output rows (flat block k)
                if rs_buf[0] is None:
                    rs_buf[0] = dram.tile([NHALF * 128, H], bf16,
                                          name="rsin")
                    rs_buf[1] = dram.tile([NHALF * BQ, H], bf16,
                                          name="rsout")
                nc.sync.dma_start(rs_buf[0][h * 128:(h + 1) * 128, :],
                                  z_t[h][:])
                if h == NHALF - 1:
                    nc.gpsimd.collective_compute(
                        "ReduceScatter", ALU.add,
                        replica_groups=[list(range(NCORE))],
                        ins=[rs_buf[0].opt()], outs=[rs_buf[1].opt()],
                    )

            BO = NHALF * BQ  # 32 output rows

            def tail():
                z32 = work.tile([BO, H], bf16, name="z32")
                nc.scalar.dma_start(z32[:, 0:256], rs_buf[1][:, 0:256])
                nc.sync.dma_start(z32[:, 256:H], rs_buf[1][:, 256:H])
                zrT = work.tile([128, HJ, BO], bf16, name="zrT")
                for hj in range(HJ):
                    tp = ps.tile([128, BO], bf16, tag="zt", bufs=2,
                                 name=f"tp_{hj}")
                    nc.tensor.transpose(
                        tp[:], z32[:, hj * 128:(hj + 1) * 128], eye_sb[:])
                    nc.scalar.activation(zrT[:, hj, :], tp[:], AF.Relu,
                                         bias=bf1t_sb[:, hj:hj + 1])
                lp = ps.tile([BO, N_OUT], f32, tag="zt", bufs=2, name="lp")
                for hj in range(HJ):
                    nc.tensor.matmul(lp[:], zrT[:, hj, :], wf2_sb[:, hj, :],
                                     start=(hj == 0), stop=False)
                nc.tensor.matmul(lp[:], ones16[0:1, :], bf2_sb[:],
                                 start=False, stop=True)
                # logits are small enough that exp() needs no max-shift
                ex = work.tile([BO, N_OUT], f32, name="ex")
                nc.scalar.activation(ex[:], lp[:], AF.Exp)
                sm = work.tile([BO, 1], f32, name="sm")
                nc.vector.tensor_reduce(sm[:], ex[:], axis=AX.X, op=ALU.add)
                rc = work.tile([BO, 1], f32, name="rc")
                nc.vector.reciprocal(rc[:], sm[:])
                ob = work.tile([BO, N_OUT], f32, name="ob")
                nc.vector.tensor_scalar(ob[:], ex[:], rc[0:BO, 0:1], None,
                                        ALU.mult)
                nc.scalar.dma_start(out_d[:], ob[:])

            # ---- pipelined emission schedule -----------------------------
            elu1(0)
            elu1(1)
            # wf1 (3.7 MB) rides the scalar HWDGE queue here: issues after
            # the elu1 ACT ops, well clear of the startup HBM rush, and
            # lands long before FC1 needs it.
            nc.scalar.dma_start(wf1_sb[:],
                                wf1_d[:].rearrange("(s p) h -> p s h", p=128))
            conv2(0)
            a2a_send(0)
            conv2(1)
            a2a_send(1)
            a2a_recv(0)
            mix_fc1(0)
            rs_push(0)
            a2a_recv(1)
            mix_fc1(1)
            rs_push(1)
            tail()

    _install_wait_splitter(nc)
    return nc


_NC_CACHE = None


def _get_program():
    global _NC_CACHE
    if _NC_CACHE is None:
        _NC_CACHE = _build_program()
    return _NC_CACHE


# ---------------------------------------------------------------------------
def _batch_perm():
    # identity: half 0 = batches 0..128 (outputs of cores 0-3), half 1 =
    # 128..256 (cores 4-7); the merged ReduceScatter's flat 32-row block
    # k is then exactly core k's output slice.
    return np.arange(B)


def _prep_inputs(x, edge_row, edge_col, edge_val, W1, b1, W2, b2,
                 Wf1, bf1, Wf2, bf2):
    import ml_dtypes
    f = np.float32
    bf = ml_dtypes.bfloat16
    A = np.zeros((N, N), f)
    np.add.at(A, (np.asarray(edge_row), np.asarray(edge_col)),
              np.asarray(edge_val, f))
    AT = np.ascontiguousarray(A.T)                                  # [k, m]
    # m-major chunks: at[p, (mo, kc, mw)] = AT[kc*112+p, mo*112+mw]
    at = np.ascontiguousarray(
        AT.reshape(KN, P, KN, P).transpose(1, 2, 0, 3).reshape(
            P, KN * KN * P)).astype(bf)

    XT = np.ascontiguousarray(np.asarray(x, f)[:, :, 0].T)          # [N, B]
    XT = XT[:, _batch_perm()]
    xt = np.ascontiguousarray(
        XT.reshape(KN, P, B).transpose(1, 0, 2).reshape(P, KN * B)).astype(bf)

    W1 = np.asarray(W1, f); b1 = np.asarray(b1, f)
    W2 = np.asarray(W2, f); b2 = np.asarray(b2, f)
    Wf1 = np.asarray(Wf1, f); bf1 = np.asarray(bf1, f)
    Wf2 = np.asarray(Wf2, f); bf2 = np.asarray(bf2, f)

    # mix weight: lhsT[(ng,c),(ng',c')] = delta(ng,ng') * W2[c,c']
    w2k = np.kron(np.eye(NG, dtype=f), W2).astype(bf)               # [128,128]
    b2k = np.tile(b2, NG).reshape(128, 1).astype(f)

    # FC1: core k's K-chunk s holds flat rows (n=112k+ng*28+s)*C + c at
    # partition p = ng*C + c; rows for pad nodes (n >= 784) are zero.
    NPAD = P * NCORE
    Wf1_pad = np.zeros((NPAD, C, H), f)
    Wf1_pad[:N] = Wf1.reshape(N, C, H)

    bf1t = np.ascontiguousarray(bf1.reshape(HJ, 128).T)             # [128, HJ]
    wf2_l = np.ascontiguousarray(
        Wf2.reshape(HJ, 128, N_OUT).transpose(1, 0, 2).reshape(
            128, HJ * N_OUT)).astype(bf)
    bf2_l = bf2.reshape(1, N_OUT).astype(bf)
    eye16 = np.eye(32, dtype=f).astype(bf)

    in_maps = []
    for k in range(NCORE):
        wb = np.concatenate([W1[0, k * CPC:(k + 1) * CPC],
                             b1[k * CPC:(k + 1) * CPC]]).reshape(1, 2 * CPC)
        # ng-major: chunk s, partition (ng, c) holds node ng*28+s
        wk = Wf1_pad[k * P:(k + 1) * P].reshape(NG, NS, C, H)
        wf1_l = np.ascontiguousarray(
            wk.transpose(1, 0, 2, 3).reshape(NS * 128, H)).astype(bf)
        in_maps.append({
            "at": at, "xt": xt, "wf1": wf1_l,
            "wb": np.ascontiguousarray(wb.astype(f)),
            "w2k": w2k, "b2k": b2k,
            "bf1t": bf1t, "wf2": wf2_l, "bf2": bf2_l, "eye16": eye16,
        })
    return in_maps


def _install_ntff_hook():
    """Self-contained NTFF profile hook for trace=True under axon (the
    image's antenv package lacks axon_hooks; without this, tracing is
    skipped and exec_time_ns comes back None)."""
    import sys
    import types
    if "antenv.axon_hooks" in sys.modules:
        return
    try:
        import antenv
        from trn_agent_boot.trn_boot import _ntff_profile_via_ctypes
        mod = types.ModuleType("antenv.axon_hooks")
        mod._hook = _ntff_profile_via_ctypes("/opt/axon/libaxon_pjrt.so")
        mod.set_axon_ntff_profile_hook = lambda h: setattr(mod, "_hook", h)
        mod.get_axon_ntff_profile_hook = lambda: mod._hook
        sys.modules["antenv.axon_hooks"] = mod
        antenv.axon_hooks = mod
    except Exception:
        pass


def kernel(x, edge_row, edge_col, edge_val, W1, b1, W2, b2,
           Wf1, bf1, Wf2, bf2, **kw):
    if kw.get("trace"):
        _install_ntff_hook()
    nc = _get_program()
    in_maps = _prep_inputs(x, edge_row, edge_col, edge_val, W1, b1, W2, b2,
                           Wf1, bf1, Wf2, bf2)
    res = run_bass_kernel_spmd(nc, in_maps, list(range(NCORE)), **kw)
    out = np.concatenate([res.results[k]["out"] for k in range(NCORE)], axis=0)
    if kw.get("trace"):
        kernel.last_exec_time_ns = res.exec_time_ns
    return out.astype(np.float32)


# revision 37
# speedup vs baseline: 1.0429x; 1.0429x over previous
"""GNN message-passing net on 8 Trainium2 cores — batch-pipelined v2.

Reference: x:[256,784,1] -> h1 = elu(spmm(x)@W1+b1) -> h2 = elu(spmm(h1)@W2+b2)
-> flat[B, N*C] -> relu(flat@Wf1+bf1) -> softmax(z@Wf2+bf2).

Strategy (all matmul operands bf16, fp32 PSUM accumulation):
  * Densify A (784x784, ~1% nz) on host; spmm becomes dense matmuls.
  * conv1 outer product: y = A @ X^T [784,256] replicated per core;
    h1_c = elu(W1[c]*y+b1[c]) for this core's 4 channels (elu via
    Exp/Relu ACT passes with fused scale/bias + 2 DVE ops).
  * conv2 channel-sharded: core k computes A @ h1_c for channels
    4k..4k+4, batch-half free dim.
  * AllToAll (per batch half) reshards channel->node; W2 mix as a
    kron(I4,W2) stationary matmul; +b2, elu.
  * FC1 K-sharded over nodes with h2 chunks as the STATIONARY operand:
    z_half[128b, 512h] accumulates 28 chunks per batch half.
  * One merged ReduceScatter over [256,512] (identity batch split:
    half 0 = cores 0-3's outputs): flat 32-row block k IS core k's
    output slice. Tail: PE-transpose z, +bf1 relu, FC2, softmax, once.
  * The two batch halves are software-pipelined: collectives (TOPSP/
    SDMA silicon) overlap compute; the conv side finishes well before
    the collective runtime's one-time init barrier (~40us, absorbed by
    a tiny AllGather alongside its ~12us first-op tax) lifts.
"""
import json

import numpy as np

import concourse.bass as bass
import concourse.mybir as mybir
import concourse.tile as tile
from concourse.bass_utils import run_bass_kernel_spmd

B, N, F, E = 256, 784, 1, 6272
C, H, N_OUT = 32, 512, 10
NCORE = 8
CPC = C // NCORE      # 4 channels per core in conv2
P = 112               # 784 = 7 * 112
KN = N // P           # 7 node chunks
NG = 4                # node groups packed into partitions for the mix
NS = P // NG          # 28 nodes per group per core
HJ = H // 128         # 4 h chunks
BH = B // 2           # batch half (pipeline stage)
BQ = 16               # output rows per core per half
NHALF = 2

f32 = mybir.dt.float32
bf16 = mybir.dt.bfloat16
AF = mybir.ActivationFunctionType
ALU = mybir.AluOpType
AX = mybir.AxisListType


# ---------------------------------------------------------------------------
# BIR post-pass: this walrus build rejects instructions with >1 sync-wait;
# split extras onto standalone EventSemaphore instructions (same engine,
# inserted just before, so the engine stream stalls identically).
def _split_waits(bir: dict, max_waits: int = 1) -> dict:
    n = [0]
    for fn in bir.get("functions", []):
        for blk in fn.get("blocks", []):
            out = []
            for ins in blk.get("instructions", []):
                si = ins.get("sync_info") or {}
                waits = si.get("on_wait") or []
                if len(waits) > max_waits:
                    for w in waits[max_waits:]:
                        n[0] += 1
                        out.append({
                            "name": f"I-waitsplit-{n[0]}",
                            "opcode": "EventSemaphore",
                            "engine": ins["engine"],
                            "ins": [], "outs": [],
                            **({"debug": ins["debug"]} if "debug" in ins else {}),
                            "sync_info": {"on_update": [], "on_wait": [w]},
                        })
                    si = dict(si)
                    si["on_wait"] = waits[:max_waits]
                    ins = dict(ins)
                    ins["sync_info"] = si
                out.append(ins)
            blk["instructions"] = out
    return bir


def _install_wait_splitter(nc):
    orig = nc.to_json_bytes
    nc.to_json_bytes = lambda: json.dumps(_split_waits(json.loads(orig()))).encode()


# ---------------------------------------------------------------------------
def _build_program():
    nc = bass.Bass(num_devices=NCORE)

    at_d = nc.dram_tensor("at", [P, KN * N], bf16, kind="ExternalInput")
    xt_d = nc.dram_tensor("xt", [P, KN * B], bf16, kind="ExternalInput")
    wf1_d = nc.dram_tensor("wf1", [NS * 128, H], bf16, kind="ExternalInput")
    wb_d = nc.dram_tensor("wb", [1, 2 * CPC], f32, kind="ExternalInput")
    w2k_d = nc.dram_tensor("w2k", [128, 128], bf16, kind="ExternalInput")
    b2k_d = nc.dram_tensor("b2k", [128, 1], f32, kind="ExternalInput")
    bf1t_d = nc.dram_tensor("bf1t", [128, HJ], f32, kind="ExternalInput")
    wf2_d = nc.dram_tensor("wf2", [128, HJ * N_OUT], bf16, kind="ExternalInput")
    bf2_d = nc.dram_tensor("bf2", [1, N_OUT], bf16, kind="ExternalInput")
    eye_d = nc.dram_tensor("eye16", [32, 32], bf16, kind="ExternalInput")
    out_d = nc.dram_tensor("out", [NHALF * BQ, N_OUT], f32, kind="ExternalOutput")

    with tile.TileContext(nc) as tc:
        with (
            tc.tile_pool(name="big", bufs=1) as big,
            tc.tile_pool(name="work", bufs=1) as work,
            tc.tile_pool(name="ps", bufs=1, space="PSUM") as ps,
            tc.tile_pool(name="dram", bufs=1, space="DRAM") as dram,
        ):
            # ---- resident tiles ------------------------------------------
            # at_sb is m-major: [p, mo, kc, mw] = A^T[kc*112+p, mo*112+mw]
            at_sb = big.tile([P, KN, KN, P], bf16)
            xt_sb = big.tile([P, KN, B], bf16)
            wf1_sb = big.tile([128, NS, H], bf16)
            y_sb = big.tile([P, KN, B], f32)
            wb_sb = work.tile([1, 2 * CPC], f32)
            w2k_sb = work.tile([128, 128], bf16)
            b2k_sb = work.tile([128, 1], f32)
            bf1t_sb = work.tile([128, HJ], f32)
            wf2_sb = work.tile([128, HJ, N_OUT], bf16)
            bf2_sb = work.tile([1, N_OUT], bf16)
            eye_sb = work.tile([32, 32], bf16)
            ones16 = work.tile([1, NHALF * BQ], bf16)
            ones_f = work.tile([1, 128], f32)
            warm_sb = work.tile([128, 512], bf16)
            zpad_sb = work.tile([P, CPC, BH], bf16)
            dummy_sb = work.tile([1, 16], f32)
            wband = work.tile([128, 2 * CPC], f32)

            # ---- t0: memsets + input loads (at/xt critical on sync; the
            # small weights go on gpsimd; the big wf1 is deferred to the
            # scalar queue later — it is only needed by FC1) ---------------
            nc.vector.memset(ones16[:], 1.0)
            nc.vector.memset(ones_f[:], 1.0)
            nc.vector.memset(warm_sb[:], 0.03)
            nc.vector.memset(zpad_sb[:], 0.0)
            nc.vector.memset(dummy_sb[:], 0.0)

            # tiny AllGather absorbs the one-time collective-runtime init
            # (~40us barrier + ~12us first-op tax) while compute proceeds
            dummy_in = dram.tile([1, 16], f32)
            dummy_out = dram.tile([NCORE, 16], f32, addr_space="Shared")
            nc.gpsimd.dma_start(dummy_in[:], dummy_sb[:])
            nc.gpsimd.collective_compute(
                "AllGather", ALU.bypass,
                replica_groups=[list(range(NCORE))],
                ins=[dummy_in.opt()], outs=[dummy_out.opt()],
            )

            nc.sync.dma_start(xt_sb[:],
                              xt_d[:].rearrange("p (k b) -> p k b", k=KN))
            at_ap = at_d[:].rearrange("p (m k w) -> p m k w", m=KN, k=KN)
            for mo in range(KN):
                nc.sync.dma_start(at_sb[:, mo, :, :], at_ap[:, mo, :, :])
            nc.gpsimd.dma_start(wb_sb[:], wb_d[:])
            nc.gpsimd.dma_start(w2k_sb[:], w2k_d[:])
            nc.gpsimd.dma_start(b2k_sb[:], b2k_d[:])
            nc.gpsimd.dma_start(bf1t_sb[:], bf1t_d[:])
            nc.gpsimd.dma_start(wf2_sb[:],
                                wf2_d[:].rearrange("p (j o) -> p j o", j=HJ))
            nc.gpsimd.dma_start(bf2_sb[:], bf2_d[:])
            nc.gpsimd.dma_start(eye_sb[:], eye_d[:])

            # ---- TE warmup (keeps HAM at 8/8 while inputs load) ----------
            for w in range(20):
                wp = ps.tile([128, 8, BH], f32, tag="pm", bufs=2,
                              name=f"warm{w}")
                nc.tensor.matmul(wp[:, 0:4, :], warm_sb[:, 0:128],
                                 warm_sb[:])

            # ---- broadcast W1/b1 channel scalars across partitions -------
            ps_bc = ps.tile([128, BQ], f32, tag="zt", bufs=2)
            nc.tensor.matmul(ps_bc[:, 0:2 * CPC], ones_f[0:1, 0:128], wb_sb[:])
            nc.vector.tensor_copy(wband[:], ps_bc[:, 0:2 * CPC])

            # ---- conv1 (both halves, free=256) ---------------------------
            for mo in range(KN):
                yp = ps.tile([P, B], f32, tag="mm1", bufs=2,
                             name=f"yp{mo}")
                for kc in range(KN):
                    nc.tensor.matmul(
                        yp[:], at_sb[:, mo, kc, :], xt_sb[:, kc, :],
                        start=(kc == 0), stop=(kc == KN - 1),
                    )
                nc.vector.tensor_copy(y_sb[:, mo, :], yp[:])

            # ---- per-half stage builders ---------------------------------
            h1_t = [None] * NHALF
            out2_t = [None] * NHALF
            a2ain_t = [None] * NHALF
            a2aout_t = [None] * NHALF
            r_t = [None] * NHALF
            h2_t = [None] * NHALF
            z_t = [None] * NHALF

            def elu1(h):
                h1 = big.tile([P, CPC, KN, BH], bf16, tag="h1", bufs=2,
                              name=f"h1_{h}")
                h1_t[h] = h1
                ysl = y_sb[:, :, h * BH:(h + 1) * BH]
                for c in range(CPC):
                    sc = wband[0:P, c:c + 1]
                    bi = wband[0:P, CPC + c:CPC + c + 1]
                    e = work.tile([P, KN, BH], bf16, tag="ew", bufs=3,
                                  name=f"e1_{h}{c}")
                    r = work.tile([P, KN, BH], bf16, tag="rw", bufs=3,
                                  name=f"r1_{h}{c}")
                    nc.scalar.activation(e[:], ysl, AF.Exp, bias=bi, scale=sc)
                    if c % 2:
                        # relu path on DVE to balance the engines
                        t = work.tile([P, KN, BH], f32, tag="tw", bufs=2,
                                      name=f"t1_{h}{c}")
                        nc.vector.tensor_scalar(t[:], ysl, sc, bi,
                                                ALU.mult, ALU.add)
                        nc.vector.tensor_scalar(r[:], t[:], 0.0, None,
                                                ALU.max)
                    else:
                        nc.scalar.activation(r[:], ysl, AF.Relu,
                                             bias=bi, scale=sc)
                    nc.vector.tensor_scalar(e[:], e[:], 1.0, -1.0,
                                            ALU.min, ALU.add)
                    nc.vector.tensor_tensor(h1[:, c, :, :], e[:], r[:],
                                            ALU.add)

            def conv2(h):
                h1 = h1_t[h]
                # [p, (mo, cl), b] so the a2a send merges (mo, cl) cleanly
                out2 = big.tile([P, KN * CPC, BH], bf16, tag="o2", bufs=2,
                                name=f"out2_{h}")
                out2_t[h] = out2
                for mo in range(KN):
                    o2 = ps.tile([P, CPC, BH], f32, tag="mm1", bufs=2,
                                 name=f"o2p_{h}{mo}")
                    for kc in range(KN):
                        nc.tensor.matmul(
                            o2[:], at_sb[:, mo, kc, :], h1[:, :, kc, :],
                            start=(kc == 0), stop=(kc == KN - 1),
                        )
                    dst = out2[:, mo * CPC:(mo + 1) * CPC, :]
                    if mo % 2:
                        nc.scalar.copy(dst, o2[:])
                    else:
                        nc.vector.tensor_copy(dst, o2[:])

            def a2a_send(h):
                a2ain = dram.tile([NCORE, CPC * P, BH], bf16, tag="a2ai",
                                  bufs=2, name=f"a2ain_{h}")
                a2aout = dram.tile([NCORE, CPC * P, BH], bf16, tag="a2ao",
                                   bufs=2, name=f"a2aout_{h}")
                a2ain_t[h], a2aout_t[h] = a2ain, a2aout
                dst = a2ain[0:KN].rearrange("j (cl p) b -> p (j cl) b",
                                            cl=CPC)
                nc.sync.dma_start(dst[:, 0:16, :], out2_t[h][:, 0:16, :])
                nc.sync.dma_start(dst[:, 16:28, :], out2_t[h][:, 16:28, :])
                nc.gpsimd.dma_start(
                    a2ain[KN].rearrange("(cl p) b -> p cl b", cl=CPC),
                    zpad_sb[:])
                nc.gpsimd.collective_compute(
                    "AllToAll", ALU.bypass,
                    replica_groups=[list(range(NCORE))],
                    ins=[a2ain.opt()], outs=[a2aout.opt()],
                )

            def a2a_recv(h):
                r_sb = big.tile([128, NS, BH], bf16, tag="r", bufs=2,
                                name=f"r_{h}")
                r_t[h] = r_sb
                sap = a2aout_t[h][:].rearrange(
                    "k (cl ng s) b -> ng (k cl) s b", cl=CPC, ng=NG)
                for ng in range(NG):
                    eng = nc.gpsimd if ng < 2 else nc.sync
                    eng.dma_start(r_sb[ng * C:(ng + 1) * C, 0:8, :],
                                  sap[ng][:, 0:8, :])
                for ng in range(NG):
                    eng = nc.gpsimd if ng < 2 else nc.sync
                    eng.dma_start(r_sb[ng * C:(ng + 1) * C, 8:NS, :],
                                  sap[ng][:, 8:NS, :])

            def mix_fc1(h):
                r_sb = r_t[h]
                h2 = big.tile([128, NS, BH], bf16, tag="h2", bufs=2,
                              name=f"h2_{h}")
                h2_t[h] = h2
                zp = ps.tile([128, H], f32, tag="zt", bufs=2, name=f"zp_{h}")

                def mix_pair(g):
                    # two 4-slot mix blocks share one 2-bank psum tile so
                    # the elu ops run at FD=1024 (halves instr overhead)
                    s0 = 8 * g
                    sw = min(8, NS - s0)
                    pm = ps.tile([128, 8, BH], f32, tag="pm", bufs=2,
                                 name=f"pm_{h}{g}")
                    nc.tensor.matmul(pm[:, 0:4, :], w2k_sb[:],
                                     r_sb[:, s0:s0 + 4, :])
                    if sw == 8:
                        nc.tensor.matmul(pm[:, 4:8, :], w2k_sb[:],
                                         r_sb[:, s0 + 4:s0 + 8, :])
                    pv = pm[:, 0:sw, :]
                    e = work.tile([128, 8, BH], bf16, tag="e2", bufs=3,
                                  name=f"e2_{h}{g}")
                    r = work.tile([128, 8, BH], bf16, tag="r2", bufs=3,
                                  name=f"r2_{h}{g}")
                    nc.scalar.activation(e[:, 0:sw, :], pv, AF.Exp,
                                         bias=b2k_sb[:, 0:1])
                    if g % 2:
                        nc.vector.tensor_scalar(r[:, 0:sw, :], pv,
                                                b2k_sb[:, 0:1], 0.0,
                                                ALU.add, ALU.max)
                    else:
                        nc.scalar.activation(r[:, 0:sw, :], pv, AF.Relu,
                                             bias=b2k_sb[:, 0:1])
                    nc.vector.tensor_scalar(e[:, 0:sw, :], e[:, 0:sw, :],
                                            1.0, -1.0, ALU.min, ALU.add)
                    nc.vector.tensor_tensor(h2[:, s0:s0 + sw, :],
                                            e[:, 0:sw, :], r[:, 0:sw, :],
                                            ALU.add)

                def fc1_run(s0, s1):
                    for s in range(s0, s1):
                        nc.tensor.matmul(
                            zp[:], h2[:, s, :], wf1_sb[:, s, :],
                            start=(s == 0), stop=(s == NS - 1),
                        )

                mix_pair(0)          # s 0..8
                mix_pair(1)          # s 8..16
                fc1_run(0, 8)
                mix_pair(2)          # s 16..24
                fc1_run(8, 16)
                mix_pair(3)          # s 24..28
                fc1_run(16, 24)
                fc1_run(24, 28)

                z_sb = work.tile([128, H], bf16, tag="z", bufs=2,
                                 name=f"z_{h}")
                z_t[h] = z_sb
                nc.scalar.copy(z_sb[:], zp[:])

            rs_buf = [None, None]

            def rs_push(h):
                # both halves land in one [256, H] buffer; one RS hands
                # core k its 32 output rows (flat block k)
                if rs_buf[0] is None:
                    rs_buf[0] = dram.tile([NHALF * 128, H], bf16,
                                          name="rsin")
                    rs_buf[1] = dram.tile([NHALF * BQ, H], bf16,
                                          name="rsout")
                nc.sync.dma_start(rs_buf[0][h * 128:(h + 1) * 128, :],
                                  z_t[h][:])
                if h == NHALF - 1:
                    nc.gpsimd.collective_compute(
                        "ReduceScatter", ALU.add,
                        replica_groups=[list(range(NCORE))],
                        ins=[rs_buf[0].opt()], outs=[rs_buf[1].opt()],
                    )

            BO = NHALF * BQ  # 32 output rows

            def tail():
                z32 = work.tile([BO, H], bf16, name="z32")
                nc.scalar.dma_start(z32[:, 0:256], rs_buf[1][:, 0:256])
                nc.sync.dma_start(z32[:, 256:H], rs_buf[1][:, 256:H])
                zrT = work.tile([128, HJ, BO], bf16, name="zrT")
                for hj in range(HJ):
                    tp = ps.tile([128, BO], bf16, tag="zt", bufs=2,
                                 name=f"tp_{hj}")
                    nc.tensor.transpose(
                        tp[:], z32[:, hj * 128:(hj + 1) * 128], eye_sb[:])
                    nc.scalar.activation(zrT[:, hj, :], tp[:], AF.Relu,
                                         bias=bf1t_sb[:, hj:hj + 1])
                lp = ps.tile([BO, N_OUT], f32, tag="zt", bufs=2, name="lp")
                for hj in range(HJ):
                    nc.tensor.matmul(lp[:], zrT[:, hj, :], wf2_sb[:, hj, :],
                                     start=(hj == 0), stop=False)
                nc.tensor.matmul(lp[:], ones16[0:1, :], bf2_sb[:],
                                 start=False, stop=True)
                # logits are small enough that exp() needs no max-shift
                ex = work.tile([BO, N_OUT], f32, name="ex")
                nc.scalar.activation(ex[:], lp[:], AF.Exp)
                sm = work.tile([BO, 1], f32, name="sm")
                nc.vector.tensor_reduce(sm[:], ex[:], axis=AX.X, op=ALU.add)
                rc = work.tile([BO, 1], f32, name="rc")
                nc.vector.reciprocal(rc[:], sm[:])
                ob = work.tile([BO, N_OUT], f32, name="ob")
                nc.vector.tensor_scalar(ob[:], ex[:], rc[0:BO, 0:1], None,
                                        ALU.mult)
                nc.scalar.dma_start(out_d[:], ob[:])

            # ---- pipelined emission schedule -----------------------------
            elu1(0)
            elu1(1)
            # wf1 (3.7 MB) rides the scalar HWDGE queue here: issues after
            # the elu1 ACT ops, well clear of the startup HBM rush, and
            # lands long before FC1 needs it.
            nc.scalar.dma_start(wf1_sb[:],
                                wf1_d[:].rearrange("(s p) h -> p s h", p=128))
            conv2(0)
            a2a_send(0)
            conv2(1)
            a2a_send(1)
            a2a_recv(0)
            mix_fc1(0)
            rs_push(0)
            a2a_recv(1)
            mix_fc1(1)
            rs_push(1)
            tail()

    _install_wait_splitter(nc)
    return nc


_NC_CACHE = None


def _get_program():
    global _NC_CACHE
    if _NC_CACHE is None:
        _NC_CACHE = _build_program()
    return _NC_CACHE


# ---------------------------------------------------------------------------
def _batch_perm():
    # identity: half 0 = batches 0..128 (outputs of cores 0-3), half 1 =
    # 128..256 (cores 4-7); the merged ReduceScatter's flat 32-row block
    # k is then exactly core k's output slice.
    return np.arange(B)


def _prep_inputs(x, edge_row, edge_col, edge_val, W1, b1, W2, b2,
                 Wf1, bf1, Wf2, bf2):
    import ml_dtypes
    f = np.float32
    bf = ml_dtypes.bfloat16
    A = np.zeros((N, N), f)
    np.add.at(A, (np.asarray(edge_row), np.asarray(edge_col)),
              np.asarray(edge_val, f))
    AT = np.ascontiguousarray(A.T)                                  # [k, m]
    # m-major chunks: at[p, (mo, kc, mw)] = AT[kc*112+p, mo*112+mw]
    at = np.ascontiguousarray(
        AT.reshape(KN, P, KN, P).transpose(1, 2, 0, 3).reshape(
            P, KN * KN * P)).astype(bf)

    XT = np.ascontiguousarray(np.asarray(x, f)[:, :, 0].T)          # [N, B]
    XT = XT[:, _batch_perm()]
    xt = np.ascontiguousarray(
        XT.reshape(KN, P, B).transpose(1, 0, 2).reshape(P, KN * B)).astype(bf)

    W1 = np.asarray(W1, f); b1 = np.asarray(b1, f)
    W2 = np.asarray(W2, f); b2 = np.asarray(b2, f)
    Wf1 = np.asarray(Wf1, f); bf1 = np.asarray(bf1, f)
    Wf2 = np.asarray(Wf2, f); bf2 = np.asarray(bf2, f)

    # mix weight: lhsT[(ng,c),(ng',c')] = delta(ng,ng') * W2[c,c']
    w2k = np.kron(np.eye(NG, dtype=f), W2).astype(bf)               # [128,128]
    b2k = np.tile(b2, NG).reshape(128, 1).astype(f)

    # FC1: core k's K-chunk s holds flat rows (n=112k+ng*28+s)*C + c at
    # partition p = ng*C + c; rows for pad nodes (n >= 784) are zero.
    NPAD = P * NCORE
    Wf1_pad = np.zeros((NPAD, C, H), f)
    Wf1_pad[:N] = Wf1.reshape(N, C, H)

    bf1t = np.ascontiguousarray(bf1.reshape(HJ, 128).T)             # [128, HJ]
    wf2_l = np.ascontiguousarray(
        Wf2.reshape(HJ, 128, N_OUT).transpose(1, 0, 2).reshape(
            128, HJ * N_OUT)).astype(bf)
    bf2_l = bf2.reshape(1, N_OUT).astype(bf)
    eye16 = np.eye(32, dtype=f).astype(bf)

    in_maps = []
    for k in range(NCORE):
        wb = np.concatenate([W1[0, k * CPC:(k + 1) * CPC],
                             b1[k * CPC:(k + 1) * CPC]]).reshape(1, 2 * CPC)
        # ng-major: chunk s, partition (ng, c) holds node ng*28+s
        wk = Wf1_pad[k * P:(k + 1) * P].reshape(NG, NS, C, H)
        wf1_l = np.ascontiguousarray(
            wk.transpose(1, 0, 2, 3).reshape(NS * 128, H)).astype(bf)
        in_maps.append({
            "at": at, "xt": xt, "wf1": wf1_l,
            "wb": np.ascontiguousarray(wb.astype(f)),
            "w2k": w2k, "b2k": b2k,
            "bf1t": bf1t, "wf2": wf2_l, "bf2": bf2_l, "eye16": eye16,
        })
    return in_maps


def _install_ntff_hook():
    """Self-contained NTFF profile hook for trace=True under axon (the
    image's antenv package lacks axon_hooks; without this, tracing is
    skipped and exec_time_ns comes back None)."""
    import sys
    import types
    if "antenv.axon_hooks" in sys.modules:
        return
    try:
        import antenv
        from trn_agent_boot.trn_boot import _ntff_profile_via_ctypes
        mod = types.ModuleType("antenv.axon_hooks")
        mod._hook = _ntff_profile_via_ctypes("/opt/axon/libaxon_pjrt.so")
        mod.set_axon_ntff_profile_hook = lambda h: setattr(mod, "_hook", h)
        mod.get_axon_ntff_profile_hook = lambda: mod._hook
        sys.modules["antenv.axon_hooks"] = mod
        antenv.axon_hooks = mod
    except Exception:
        pass


def kernel(x, edge_row, edge_col, edge_val, W1, b1, W2, b2,
           Wf1, bf1, Wf2, bf2, **kw):
    if kw.get("trace"):
        _install_ntff_hook()
    nc = _get_program()
    in_maps = _prep_inputs(x, edge_row, edge_col, edge_val, W1, b1, W2, b2,
                           Wf1, bf1, Wf2, bf2)
    res = run_bass_kernel_spmd(nc, in_maps, list(range(NCORE)), **kw)
    out = np.concatenate([res.results[k]["out"] for k in range(NCORE)], axis=0)
    if kw.get("trace"):
        kernel.last_exec_time_ns = res.exec_time_ns
    return out.astype(np.float32)
